# revision 31
# baseline (speedup 1.0000x reference)
"""Multi-head self-attention (B=2, S=2048, D=1024, H=16) on 8 trn2 cores.

Sharding: core c = b*4 + g  (b = batch, g = head-group of 4 heads).

Fused single-phase pipeline per core (batch b, heads 4g..4g+3), f16 matmuls:
  - K bias dropped entirely: softmax_k((q+bq)*(k+bk)) == softmax_k((q+bq)*k)
    since per-query-constant terms cancel in softmax.
  - Q/K feature-major [128=2heads*64feat, S]; V seq-major per 128-key chunk
    [128 keys, 4 heads, 65] with a ones column for the softmax rowsum.
  - Attention unit = (head pair t, 512-query chunk): per 128-key block,
    scores via two row-tiled 64-contract matmuls (head parity p on
    partition halves; LDWEIGHTS hides behind the other parity's matmul),
    one [128,1024] exp on ACT, PV via row-disjoint bank-disjoint pairs.
  - Projections and the output projection are interleaved into the
    attention stream as fillers so they run in PE slack under the ACT wall.
  - y emitted f16; host accumulates partials in f32 and adds bo + bv@Wo.
"""

import sys

sys.path.insert(0, "/opt/trn_rl_repo")

import numpy as np

import concourse.bass as bass
import concourse.mybir as mybir
import concourse.tile as tile

F32 = mybir.dt.float32
F16 = mybir.dt.float16
AF = mybir.ActivationFunctionType

D = 1024          # d_model
S = 2048          # sequence length
HPC = 4           # heads per core
DK = 64           # head dim
E = HPC * DK      # 256 features per core
N_CORES = 8
KT = D // 128     # 8 k-subtiles over d_model
SC = S // 128     # 16 key blocks / seq chunks of 128
ET = E // 128     # 2 feature tiles == head pairs
QC = 4            # 512-query chunks
EXP_SCALE = 0.125  # 1/sqrt(dk)
Y_F16 = True       # f16 kernel output
DVE_BIAS = True    # Q bias via DVE tensor_scalar_add (else ACT identity+bias)
WARM_EXP = True    # pre-trigger exp table load


_ENGINE_OPS = {
    "InstMatmult", "InstActivation", "InstTensorCopy", "InstTensorTensor",
    "InstReciprocal", "InstTensorReduce", "InstMemset", "InstIota",
    "InstTensorScalarPtr", "InstTranspose", "InstLdweights",
    "InstDMACopy", "InstDrain", "InstNoOp", "InstDmaTransposeAnt",
}


def _legalize_matmul_waits(nc):
    """walrus allows at most 1 sync wait on engine compute instructions; Tile
    sometimes emits more. Move the excess onto EventSemaphore instructions
    (cap 2 each) placed immediately before in same-engine program order."""
    for f in nc.m.functions:
        for bb in f.blocks:
            out = []
            changed = False
            for i in bb.instructions:
                si = getattr(i, "sync_info", None)
                if (
                    type(i).__name__ in _ENGINE_OPS
                    and si is not None
                    and si.on_wait
                    and len(si.on_wait) > 1
                ):
                    waits = list(si.on_wait)
                    excess, keep = waits[:-1], waits[-1:]
                    for c in range(0, len(excess), 2):
                        ev = mybir.InstEventSemaphore(
                            name=f"{i.name}-mmw{c}", ins=[], outs=[]
                        )
                        ev.engine = i.engine
                        ev.sync_info = mybir.SyncInfo(
                            on_wait=excess[c:c + 2], on_update=[]
                        )
                        out.append(ev)
                    i.sync_info = mybir.SyncInfo(
                        on_wait=keep, on_update=list(si.on_update)
                    )
                    changed = True
                out.append(i)
            if changed:
                bb.instructions = out


def build_nc(legalize=True):
    nc = bass.Bass()

    xt = nc.dram_tensor("xt", [128, KT, S], F16, kind="ExternalInput")
    wq = nc.dram_tensor("wq", [128, KT, E], F16, kind="ExternalInput")
    wk = nc.dram_tensor("wk", [128, KT, E], F16, kind="ExternalInput")
    wv = nc.dram_tensor("wv", [128, KT, E], F16, kind="ExternalInput")
    wo = nc.dram_tensor("wo", [E, D], F16, kind="ExternalInput")
    bq = nc.dram_tensor("bq", [E], F32, kind="ExternalInput")
    y = nc.dram_tensor("y", [S, D], F16 if Y_F16 else F32, kind="ExternalOutput")

    with tile.TileContext(nc) as tc:
        with (
            tc.tile_pool(name="persist", bufs=1) as pp,
            tc.tile_pool(name="work", bufs=3) as pw,
            tc.tile_pool(name="dramB", bufs=3, space="DRAM") as dramB,
            tc.tile_pool(name="psS", bufs=2, space="PSUM") as psS,
            tc.tile_pool(name="psC", bufs=2, space="PSUM") as psC,
        ):
            # ---- persistent tiles ----
            xt_sb = pp.tile([128, KT, S], F16, tag="xt")
            wq_sb = pp.tile([128, KT, E], F16, tag="wq")
            wk_sb = pp.tile([128, KT, E], F16, tag="wk")
            wv_sb = pp.tile([128, KT, E], F16, tag="wv")
            qt_sb = [pp.tile([128, S], F16, tag=f"qt{t}", name=f"qt{t}")
                     for t in range(ET)]
            ktf = [pp.tile([128, S], F16, tag=f"kt{t}", name=f"kt{t}")
                   for t in range(ET)]
            # V per 128-key chunk, seq-major, + ones column for the rowsum
            vs = [pp.tile([128, HPC, DK + 1], F16, tag=f"vs{s}", name=f"vs{s}")
                  for s in range(SC)]
            ctx_sb = [pp.tile([128, S], F16, tag=f"ctx{t}", name=f"ctx{t}")
                      for t in range(ET)]
            wo_sb = [pp.tile([128, D], F16, tag=f"wo{t}", name=f"wo{t}")
                     for t in range(ET)]
            bq_sb = pp.tile([128, ET], F32, tag="bq")
            ones64 = pp.tile([1, DK], F16, tag="ones64")

            # input DMAs, earliest-needed first; xt split by seq half so
            # the first K/Q/V chunks (queries/keys 0:1024) land early
            nc.sync.dma_start(wk_sb, wk[:])
            nc.sync.dma_start(wq_sb, wq[:])
            nc.sync.dma_start(bq_sb, bq.rearrange("(t p) -> p t", p=128))
            nc.sync.dma_start(wv_sb, wv[:])
            # first seq quarter (feeds K(0,0)/Q(0,0)/V0-3) on the scalar
            # HWDGE ring first, in parallel with the weight loads on sync
            for k in range(KT):
                nc.scalar.dma_start(xt_sb[:, k, 0:512], xt[:, k, 0:512])
            for k in range(KT):
                nc.scalar.dma_start(xt_sb[:, k, 512:1024], xt[:, k, 512:1024])
            for k in range(KT):
                nc.sync.dma_start(xt_sb[:, k, 1024:S], xt[:, k, 1024:S])
            for t in range(ET):
                nc.sync.dma_start(wo_sb[t], wo[t * 128:(t + 1) * 128, :])

            # warm the ACT exp table while input DMAs stream
            if WARM_EXP:
                warm = pp.tile([1, 16], F32, tag="warm")
                warm2 = pp.tile([1, 16], F32, tag="warm2")
                nc.vector.memset(warm, 0.0)
                nc.scalar.activation(warm2, warm, AF.Exp)

            # ones columns of vs
            for s in range(SC):
                nc.vector.memset(vs[s][:, :, DK:DK + 1], 1.0)
            nc.vector.memset(ones64, 1.0)

            # ---- projection emitters (f16, contract 128/mm) ----
            def emit_q(t, qc):
                ps2 = psS.tile([128, 2, 512], F32, tag="sc",
                               name=f"qps{t}_{qc}")
                ps = ps2[:, 0, :]
                for k in range(KT):
                    nc.tensor.matmul(
                        ps,
                        wq_sb[:, k, t * 128:(t + 1) * 128],
                        xt_sb[:, k, qc * 512:(qc + 1) * 512],
                        start=(k == 0), stop=(k == KT - 1),
                    )
                if DVE_BIAS:
                    nc.vector.tensor_scalar_add(
                        qt_sb[t][:, qc * 512:(qc + 1) * 512], ps,
                        bq_sb[:, t:t + 1],
                    )
                else:
                    nc.scalar.activation(
                        qt_sb[t][:, qc * 512:(qc + 1) * 512], ps,
                        AF.Identity, bias=bq_sb[:, t:t + 1],
                    )

            def emit_k(t, qc):
                ps2 = psS.tile([128, 2, 512], F32, tag="sc",
                               name=f"kps{t}_{qc}")
                ps = ps2[:, 0, :]
                for k in range(KT):
                    nc.tensor.matmul(
                        ps,
                        wk_sb[:, k, t * 128:(t + 1) * 128],
                        xt_sb[:, k, qc * 512:(qc + 1) * 512],
                        start=(k == 0), stop=(k == KT - 1),
                    )
                nc.vector.tensor_copy(ktf[t][:, qc * 512:(qc + 1) * 512], ps)

            def emit_v(s):
                ps2 = psS.tile([128, 2, 512], F32, tag="sc", name=f"vps{s}")
                ps = ps2[:, 0, :]
                for k in range(KT):
                    nc.tensor.matmul(
                        ps[:, 0:E],
                        xt_sb[:, k, s * 128:(s + 1) * 128],
                        wv_sb[:, k, :],
                        start=(k == 0), stop=(k == KT - 1),
                    )
                nc.vector.tensor_copy(
                    vs[s][:, :, 0:DK],
                    ps[:, 0:E].rearrange("p (h d) -> p h d", d=DK),
                )

            # ---- output projection per 128-query chunk ----
            def emit_c(qt):
                ys = pw.tile([128, D], F16 if Y_F16 else F32, tag="ys",
                             name=f"ys{qt}", bufs=3)
                yp = psS.tile([128, 2, 512], F32, tag="sc",
                              name=f"yp{qt}")
                for n in range(2):
                    for t in range(ET):
                        nc.tensor.matmul(
                            yp[:, n, :],
                            ctx_sb[t][:, qt * 128:(qt + 1) * 128],
                            wo_sb[t][:, n * 512:(n + 1) * 512],
                            start=(t == 0), stop=(t == ET - 1),
                        )
                    nc.vector.tensor_copy(ys[:, n * 512:(n + 1) * 512],
                                          yp[:, n, :])
                nc.sync.dma_start(y[qt * 128:(qt + 1) * 128, :], ys)

            # ---- attention unit: head pair t, query chunk qc (512 q) ----
            def emit_unit(t, qc, fillers, pre=None):
                """fillers: thunks interleaved between key blocks; they run
                in PE slack while ACT exponentiates.  pre: the previous
                unit's deferred normalization (broadcast matmul + multiply),
                emitted after this unit's first scores so its input chain
                latency hides behind them."""
                q0 = qc * 512
                ctx_ps = [
                    psC.tile([DK + 1, 512], F32, tag=f"c{p}",
                             name=f"ctxps{t}_{qc}_{p}")
                    for p in range(2)
                ]
                def emit_scores(cb):
                    sc = psS.tile([128, 2, 512], F32, tag="sc",
                                  name=f"sc{t}_{qc}_{cb}")
                    # scoresT[key, q] for head parity p: 64-contract matmuls
                    # on opposite partition halves -> run concurrently and
                    # hide each other's LDWEIGHTS.
                    for p in range(2):
                        nc.tensor.matmul(
                            sc[:, p, :],
                            ktf[t][64 * p:64 * p + 64,
                                   cb * 128:(cb + 1) * 128],
                            qt_sb[t][64 * p:64 * p + 64, q0:q0 + 512],
                            start=True, stop=True,
                        )
                    return sc

                fi = 0
                sc_cur = emit_scores(0)
                if pre is not None:
                    pre()
                for cb in range(SC):
                    # software pipeline: next block's scores enter the PE
                    # queue before this block's PV so PE isn't head-of-line
                    # blocked on the exp result.
                    sc_next = emit_scores(cb + 1) if cb + 1 < SC else None
                    ex = pw.tile([128, 2, 512], F16, tag="ex",
                                 name=f"ex{t}_{qc}_{cb}", bufs=4)
                    nc.scalar.activation(
                        ex.rearrange("p m q -> p (m q)"),
                        sc_cur.rearrange("p m q -> p (m q)"),
                        AF.Exp, scale=EXP_SCALE,
                    )
                    # PV: full 128-key contraction, one matmul per head.
                    # Constant tile_position (0,0) per accumulation group
                    # (mixed row bases in one group abort on HW).
                    for p in range(2):
                        nc.tensor.matmul(
                            ctx_ps[p],
                            vs[cb][:, 2 * t + p, :],
                            ex[:, p, :],
                            start=(cb == 0), stop=(cb == SC - 1),
                        )
                    if fi < len(fillers):
                        if fillers[fi] is not None:
                            fillers[fi]()
                        fi += 1
                    sc_cur = sc_next
                while fi < len(fillers):
                    if fillers[fi] is not None:
                        fillers[fi]()
                    fi += 1

                # normalize part 1 (no PE): evict ctxT, 1/rowsum via
                # exp(-ln(x)) on ACT (both functions share one table set)
                stgs, rrows = [], []
                for p in range(2):
                    stg = pw.tile([DK + 1, 512], F32, tag=f"stg{p}",
                                  name=f"stg{t}_{qc}_{p}", bufs=2)
                    nc.vector.tensor_copy(stg, ctx_ps[p])
                    lnr = pw.tile([1, 512], F32, tag=f"ln{p}",
                                  name=f"ln{t}_{qc}_{p}", bufs=2)
                    nc.scalar.activation(lnr, stg[DK:DK + 1, :], AF.Ln)
                    rrow = pw.tile([1, 512], F16, tag=f"rr{p}",
                                   name=f"rr{t}_{qc}_{p}", bufs=2)
                    nc.scalar.activation(rrow, lnr, AF.Exp, scale=-1.0)
                    stgs.append(stg)
                    rrows.append(rrow)

                def fin():
                    # normalize part 2: broadcast 1/rowsum across 64
                    # partitions via a ones-stationary f16 matmul into the
                    # drained ctx psum bank, then scale into ctx_sb
                    for p in range(2):
                        nc.tensor.matmul(ctx_ps[p][0:DK, :], ones64,
                                         rrows[p], start=True, stop=True)
                        nc.vector.tensor_mul(
                            ctx_sb[t][64 * p:64 * p + 64, q0:q0 + 512],
                            stgs[p][0:DK, :],
                            ctx_ps[p][0:DK, :],
                        )
                return fin

            # ---- schedule ----
            # minimal lead-in: one K chunk, one Q chunk, two V chunks;
            # the rest of K(t0) streams as the first fillers (ready before
            # scores reach those key blocks)
            emit_k(0, 0)
            emit_q(0, 0)
            for s in range(5):
                emit_v(s)

            K = emit_k
            Q = emit_q
            V = emit_v
            C = emit_c
            N = None
            # filler fi of a unit runs after PV(fi): V(s) must sit at
            # fi <= s-1, K(t,qc) feeding this unit at fi <= 4qc-2.
            # C(q) chunks are delayed one full unit after norm(t1,q) and
            # placed at fi >= 4 so the reciprocal DMA chain has completed.
            f = emit_unit(0, 0,
                          [lambda: V(5), lambda: K(0, 1),       # fi 0,1
                           lambda: V(6), lambda: V(7),          # fi 2,3
                           lambda: K(0, 2),                     # fi 4
                           lambda: V(8), lambda: V(9),          # fi 5,6
                           lambda: V(10),                       # fi 7
                           lambda: K(0, 3),                     # fi 8
                           lambda: V(11), lambda: V(12),        # fi 9,10
                           lambda: V(13), lambda: V(14),        # fi 11,12
                           lambda: V(15),                       # fi 13
                           lambda: K(1, 0), lambda: Q(1, 0)])   # fi 14,15
            f = emit_unit(1, 0,
                          [lambda: K(1, 1), lambda: K(1, 2),
                           lambda: K(1, 3),
                           lambda: Q(0, 1), lambda: Q(1, 1)], pre=f)
            f = emit_unit(0, 1, [lambda: Q(0, 2), lambda: Q(1, 2)], pre=f)
            f = emit_unit(1, 1, [N, N, N, N]
                          + [lambda q=q: C(q) for q in range(0, 4)]
                          + [lambda: Q(0, 3)], pre=f)
            f = emit_unit(0, 2, [lambda: Q(1, 3), N, N, N]
                          + [lambda q=q: C(q) for q in range(4, 8)], pre=f)
            f = emit_unit(1, 2, [], pre=f)
            f = emit_unit(0, 3, [N, N, N, N]
                          + [lambda q=q: C(q) for q in range(8, 12)], pre=f)
            f = emit_unit(1, 3, [], pre=f)
            f()
            for q in range(12, 16):
                emit_c(q)

    if legalize:
        _legalize_matmul_waits(nc)
    return nc


_NC_CACHE = None


def _get_nc():
    global _NC_CACHE
    if _NC_CACHE is None:
        _NC_CACHE = build_nc()
    return _NC_CACHE


def make_in_maps(inputs):
    x = np.asarray(inputs["x"], dtype=np.float32)
    Wq = np.asarray(inputs["Wq"], dtype=np.float32)
    Wk = np.asarray(inputs["Wk"], dtype=np.float32)
    Wv = np.asarray(inputs["Wv"], dtype=np.float32)
    Wo = np.asarray(inputs["Wo"], dtype=np.float32)
    bq = np.asarray(inputs["bq"], dtype=np.float32)

    f16 = np.float16
    in_maps = []
    for c in range(N_CORES):
        b, g = c // 4, c % 4
        sl = slice(g * E, (g + 1) * E)

        # [1024, n] -> [128, KT, n]
        def kfold(a):
            return np.ascontiguousarray(
                a.reshape(KT, 128, -1).transpose(1, 0, 2))
        in_maps.append({
            "xt": kfold(x[b].T.reshape(D, S)).astype(f16),
            "wq": kfold(Wq[:, sl]).astype(f16),
            "wk": kfold(Wk[:, sl]).astype(f16),
            "wv": kfold(Wv[:, sl]).astype(f16),
            "wo": np.ascontiguousarray(Wo[sl, :]).astype(f16),
            "bq": np.ascontiguousarray(bq[sl]),
        })
    return in_maps


def kernel(x, Wq, bq, Wk, bk, Wv, bv, Wo, bo):
    from concourse.bass_utils import run_bass_kernel_spmd

    x = np.asarray(x, dtype=np.float32)
    Wo = np.asarray(Wo, dtype=np.float32)
    bv = np.asarray(bv, dtype=np.float32)
    bo = np.asarray(bo, dtype=np.float32)

    B = x.shape[0]
    nc = _get_nc()
    in_maps = make_in_maps({
        "x": x, "Wq": Wq, "Wk": Wk, "Wv": Wv, "Wo": Wo, "bq": bq,
    })

    res = run_bass_kernel_spmd(nc, in_maps, core_ids=list(range(N_CORES)))

    bias_total = bo + bv @ Wo  # [D]
    out = np.zeros((B, S, D), dtype=np.float32)
    for c in range(N_CORES):
        out[c // 4] += np.asarray(res.results[c]["y"], dtype=np.float32)
    out += bias_total[None, None, :]
    return out


# revision 32
# speedup vs baseline: 1.0223x; 1.0223x over previous
"""Multi-head self-attention (B=2, S=2048, D=1024, H=16) on 8 trn2 cores.

Sharding: core c = b*4 + g  (b = batch, g = head-group of 4 heads).

Fused single-phase pipeline per core (batch b, heads 4g..4g+3), f16 matmuls:
  - K bias dropped entirely: softmax_k((q+bq)*(k+bk)) == softmax_k((q+bq)*k)
    since per-query-constant terms cancel in softmax.
  - Q/K feature-major [128=2heads*64feat, S]; V seq-major per 128-key chunk
    [128 keys, 4 heads, 65] with a ones column for the softmax rowsum.
  - Attention unit = (head pair t, 512-query chunk): per 128-key block,
    scores via two row-tiled 64-contract matmuls (head parity p on
    partition halves; LDWEIGHTS hides behind the other parity's matmul),
    one [128,1024] exp on ACT, PV via row-disjoint bank-disjoint pairs.
  - Projections and the output projection are interleaved into the
    attention stream as fillers so they run in PE slack under the ACT wall.
  - y emitted f16; host accumulates partials in f32 and adds bo + bv@Wo.
"""

import sys

sys.path.insert(0, "/opt/trn_rl_repo")

import numpy as np

import concourse.bass as bass
import concourse.mybir as mybir
import concourse.tile as tile

F32 = mybir.dt.float32
F16 = mybir.dt.float16
AF = mybir.ActivationFunctionType

D = 1024          # d_model
S = 2048          # sequence length
HPC = 4           # heads per core
DK = 64           # head dim
E = HPC * DK      # 256 features per core
N_CORES = 8
KT = D // 128     # 8 k-subtiles over d_model
SC = S // 128     # 16 key blocks / seq chunks of 128
ET = E // 128     # 2 feature tiles == head pairs
QC = 4            # 512-query chunks
EXP_SCALE = 0.125  # 1/sqrt(dk)
Y_F16 = True       # f16 kernel output
DVE_BIAS = True    # Q bias via DVE tensor_scalar_add (else ACT identity+bias)
WARM_EXP = True    # pre-trigger exp table load


_ENGINE_OPS = {
    "InstMatmult", "InstActivation", "InstTensorCopy", "InstTensorTensor",
    "InstReciprocal", "InstTensorReduce", "InstMemset", "InstIota",
    "InstTensorScalarPtr", "InstTranspose", "InstLdweights",
    "InstDMACopy", "InstDrain", "InstNoOp", "InstDmaTransposeAnt",
}


def _legalize_matmul_waits(nc):
    """walrus allows at most 1 sync wait on engine compute instructions; Tile
    sometimes emits more. Move the excess onto EventSemaphore instructions
    (cap 2 each) placed immediately before in same-engine program order."""
    for f in nc.m.functions:
        for bb in f.blocks:
            out = []
            changed = False
            for i in bb.instructions:
                si = getattr(i, "sync_info", None)
                if (
                    type(i).__name__ in _ENGINE_OPS
                    and si is not None
                    and si.on_wait
                    and len(si.on_wait) > 1
                ):
                    waits = list(si.on_wait)
                    excess, keep = waits[:-1], waits[-1:]
                    for c in range(0, len(excess), 2):
                        ev = mybir.InstEventSemaphore(
                            name=f"{i.name}-mmw{c}", ins=[], outs=[]
                        )
                        ev.engine = i.engine
                        ev.sync_info = mybir.SyncInfo(
                            on_wait=excess[c:c + 2], on_update=[]
                        )
                        out.append(ev)
                    i.sync_info = mybir.SyncInfo(
                        on_wait=keep, on_update=list(si.on_update)
                    )
                    changed = True
                out.append(i)
            if changed:
                bb.instructions = out


def build_nc(legalize=True):
    nc = bass.Bass()

    xt = nc.dram_tensor("xt", [128, KT, S], F16, kind="ExternalInput")
    wq = nc.dram_tensor("wq", [128, KT, E], F16, kind="ExternalInput")
    wk = nc.dram_tensor("wk", [128, KT, E], F16, kind="ExternalInput")
    wv = nc.dram_tensor("wv", [128, KT, E], F16, kind="ExternalInput")
    wo = nc.dram_tensor("wo", [E, D], F16, kind="ExternalInput")
    bq = nc.dram_tensor("bq", [E], F32, kind="ExternalInput")
    y = nc.dram_tensor("y", [S, D], F16 if Y_F16 else F32, kind="ExternalOutput")

    with tile.TileContext(nc) as tc:
        with (
            tc.tile_pool(name="persist", bufs=1) as pp,
            tc.tile_pool(name="work", bufs=3) as pw,
            tc.tile_pool(name="dramB", bufs=3, space="DRAM") as dramB,
            tc.tile_pool(name="psS", bufs=3, space="PSUM") as psS,
            tc.tile_pool(name="psC", bufs=1, space="PSUM") as psC,
        ):
            # ---- persistent tiles ----
            xt_sb = pp.tile([128, KT, S], F16, tag="xt")
            wq_sb = pp.tile([128, KT, E], F16, tag="wq")
            wk_sb = pp.tile([128, KT, E], F16, tag="wk")
            wv_sb = pp.tile([128, KT, E], F16, tag="wv")
            qt_sb = [pp.tile([128, S], F16, tag=f"qt{t}", name=f"qt{t}")
                     for t in range(ET)]
            ktf = [pp.tile([128, S], F16, tag=f"kt{t}", name=f"kt{t}")
                   for t in range(ET)]
            # V per 128-key chunk, seq-major, + ones column for the rowsum
            vs = [pp.tile([128, HPC, DK + 1], F16, tag=f"vs{s}", name=f"vs{s}")
                  for s in range(SC)]
            ctx_sb = [pp.tile([128, S], F16, tag=f"ctx{t}", name=f"ctx{t}")
                      for t in range(ET)]
            wo_sb = [pp.tile([128, D], F16, tag=f"wo{t}", name=f"wo{t}")
                     for t in range(ET)]
            bq_sb = pp.tile([128, ET], F32, tag="bq")
            ones64 = pp.tile([1, DK], F16, tag="ones64")

            # input DMAs, earliest-needed first; xt split by seq half so
            # the first K/Q/V chunks (queries/keys 0:1024) land early
            nc.sync.dma_start(wk_sb, wk[:])
            nc.sync.dma_start(wq_sb, wq[:])
            nc.sync.dma_start(bq_sb, bq.rearrange("(t p) -> p t", p=128))
            nc.sync.dma_start(wv_sb, wv[:])
            # first seq quarter (feeds K(0,0)/Q(0,0)/V0-3) on the scalar
            # HWDGE ring first, in parallel with the weight loads on sync
            for k in range(KT):
                nc.scalar.dma_start(xt_sb[:, k, 0:512], xt[:, k, 0:512])
            for k in range(KT):
                nc.scalar.dma_start(xt_sb[:, k, 512:1024], xt[:, k, 512:1024])
            for k in range(KT):
                nc.sync.dma_start(xt_sb[:, k, 1024:S], xt[:, k, 1024:S])
            for t in range(ET):
                nc.sync.dma_start(wo_sb[t], wo[t * 128:(t + 1) * 128, :])

            # warm the ACT exp table while input DMAs stream
            if WARM_EXP:
                warm = pp.tile([1, 16], F32, tag="warm")
                warm2 = pp.tile([1, 16], F32, tag="warm2")
                nc.vector.memset(warm, 0.0)
                nc.scalar.activation(warm2, warm, AF.Exp)

            # ones columns of vs
            for s in range(SC):
                nc.vector.memset(vs[s][:, :, DK:DK + 1], 1.0)
            nc.vector.memset(ones64, 1.0)

            # ---- projection emitters (f16, contract 128/mm) ----
            def emit_q(t, qc):
                ps2 = psS.tile([128, 2, 512], F32, tag="sc",
                               name=f"qps{t}_{qc}")
                ps = ps2[:, 0, :]
                for k in range(KT):
                    nc.tensor.matmul(
                        ps,
                        wq_sb[:, k, t * 128:(t + 1) * 128],
                        xt_sb[:, k, qc * 512:(qc + 1) * 512],
                        start=(k == 0), stop=(k == KT - 1),
                    )
                if DVE_BIAS:
                    nc.vector.tensor_scalar_add(
                        qt_sb[t][:, qc * 512:(qc + 1) * 512], ps,
                        bq_sb[:, t:t + 1],
                    )
                else:
                    nc.scalar.activation(
                        qt_sb[t][:, qc * 512:(qc + 1) * 512], ps,
                        AF.Identity, bias=bq_sb[:, t:t + 1],
                    )

            def emit_k(t, qc):
                ps2 = psS.tile([128, 2, 512], F32, tag="sc",
                               name=f"kps{t}_{qc}")
                ps = ps2[:, 0, :]
                for k in range(KT):
                    nc.tensor.matmul(
                        ps,
                        wk_sb[:, k, t * 128:(t + 1) * 128],
                        xt_sb[:, k, qc * 512:(qc + 1) * 512],
                        start=(k == 0), stop=(k == KT - 1),
                    )
                nc.vector.tensor_copy(ktf[t][:, qc * 512:(qc + 1) * 512], ps)

            def emit_v(s):
                ps2 = psS.tile([128, 2, 512], F32, tag="sc", name=f"vps{s}")
                ps = ps2[:, 0, :]
                for k in range(KT):
                    nc.tensor.matmul(
                        ps[:, 0:E],
                        xt_sb[:, k, s * 128:(s + 1) * 128],
                        wv_sb[:, k, :],
                        start=(k == 0), stop=(k == KT - 1),
                    )
                nc.vector.tensor_copy(
                    vs[s][:, :, 0:DK],
                    ps[:, 0:E].rearrange("p (h d) -> p h d", d=DK),
                )

            # ---- output projection per 128-query chunk ----
            def emit_c(qt):
                ys = pw.tile([128, D], F16 if Y_F16 else F32, tag="ys",
                             name=f"ys{qt}", bufs=3)
                yp = psS.tile([128, 2, 512], F32, tag="sc",
                              name=f"yp{qt}")
                for n in range(2):
                    for t in range(ET):
                        nc.tensor.matmul(
                            yp[:, n, :],
                            ctx_sb[t][:, qt * 128:(qt + 1) * 128],
                            wo_sb[t][:, n * 512:(n + 1) * 512],
                            start=(t == 0), stop=(t == ET - 1),
                        )
                    nc.vector.tensor_copy(ys[:, n * 512:(n + 1) * 512],
                                          yp[:, n, :])
                nc.sync.dma_start(y[qt * 128:(qt + 1) * 128, :], ys)

            # ---- attention unit: head pair t, query chunk qc (512 q) ----
            def emit_unit(t, qc, fillers, pre=None):
                """fillers: thunks interleaved between key blocks; they run
                in PE slack while ACT exponentiates.  pre: the previous
                unit's deferred normalization (broadcast matmul + multiply),
                emitted after this unit's first scores so its input chain
                latency hides behind them."""
                q0 = qc * 512
                ctx_ps = [
                    psC.tile([DK + 1, 512], F32, tag=f"c{p}",
                             name=f"ctxps{t}_{qc}_{p}")
                    for p in range(2)
                ]
                def emit_scores(cb):
                    sc = psS.tile([128, 2, 512], F32, tag="sc",
                                  name=f"sc{t}_{qc}_{cb}")
                    # scoresT[key, q] for head parity p: 64-contract matmuls
                    # on opposite partition halves -> run concurrently and
                    # hide each other's LDWEIGHTS.
                    for p in range(2):
                        nc.tensor.matmul(
                            sc[:, p, :],
                            ktf[t][64 * p:64 * p + 64,
                                   cb * 128:(cb + 1) * 128],
                            qt_sb[t][64 * p:64 * p + 64, q0:q0 + 512],
                            start=True, stop=True,
                        )
                    return sc

                fi = 0
                sc_cur = emit_scores(0)
                if pre is not None:
                    pre()
                for cb in range(SC):
                    # software pipeline: next block's scores enter the PE
                    # queue before this block's PV so PE isn't head-of-line
                    # blocked on the exp result.
                    sc_next = emit_scores(cb + 1) if cb + 1 < SC else None
                    ex = pw.tile([128, 2, 512], F16, tag="ex",
                                 name=f"ex{t}_{qc}_{cb}", bufs=4)
                    nc.scalar.activation(
                        ex.rearrange("p m q -> p (m q)"),
                        sc_cur.rearrange("p m q -> p (m q)"),
                        AF.Exp, scale=EXP_SCALE,
                    )
                    # PV: full 128-key contraction, one matmul per head.
                    # Constant tile_position (0,0) per accumulation group
                    # (mixed row bases in one group abort on HW).
                    for p in range(2):
                        nc.tensor.matmul(
                            ctx_ps[p],
                            vs[cb][:, 2 * t + p, :],
                            ex[:, p, :],
                            start=(cb == 0), stop=(cb == SC - 1),
                        )
                    if fi < len(fillers):
                        if fillers[fi] is not None:
                            fillers[fi]()
                        fi += 1
                    sc_cur = sc_next
                while fi < len(fillers):
                    if fillers[fi] is not None:
                        fillers[fi]()
                    fi += 1

                # normalize part 1 (no PE): evict ctxT, 1/rowsum via
                # exp(-ln(x)) on ACT (both functions share one table set)
                stgs, rrows = [], []
                for p in range(2):
                    stg = pw.tile([DK + 1, 512], F32, tag=f"stg{p}",
                                  name=f"stg{t}_{qc}_{p}", bufs=2)
                    nc.vector.tensor_copy(stg, ctx_ps[p])
                    lnr = pw.tile([1, 512], F32, tag=f"ln{p}",
                                  name=f"ln{t}_{qc}_{p}", bufs=2)
                    nc.scalar.activation(lnr, stg[DK:DK + 1, :], AF.Ln)
                    rrow = pw.tile([1, 512], F16, tag=f"rr{p}",
                                   name=f"rr{t}_{qc}_{p}", bufs=2)
                    nc.scalar.activation(rrow, lnr, AF.Exp, scale=-1.0)
                    stgs.append(stg)
                    rrows.append(rrow)

                def fin():
                    # normalize part 2: broadcast 1/rowsum across 64
                    # partitions via a ones-stationary f16 matmul into the
                    # drained ctx psum bank, then scale into ctx_sb
                    for p in range(2):
                        nc.tensor.matmul(ctx_ps[p][0:DK, :], ones64,
                                         rrows[p], start=True, stop=True)
                        nc.vector.tensor_mul(
                            ctx_sb[t][64 * p:64 * p + 64, q0:q0 + 512],
                            stgs[p][0:DK, :],
                            ctx_ps[p][0:DK, :],
                        )
                return fin

            # ---- schedule ----
            # minimal lead-in: one K chunk, one Q chunk, two V chunks;
            # the rest of K(t0) streams as the first fillers (ready before
            # scores reach those key blocks)
            emit_k(0, 0)
            emit_q(0, 0)
            for s in range(5):
                emit_v(s)

            K = emit_k
            Q = emit_q
            V = emit_v
            C = emit_c
            N = None
            # filler fi of a unit runs after PV(fi): V(s) must sit at
            # fi <= s-1, K(t,qc) feeding this unit at fi <= 4qc-2.
            # C(q) chunks are delayed one full unit after norm(t1,q) and
            # placed at fi >= 4 so the reciprocal DMA chain has completed.
            f = emit_unit(0, 0,
                          [lambda: V(5), lambda: K(0, 1),       # fi 0,1
                           lambda: V(6), lambda: V(7),          # fi 2,3
                           lambda: K(0, 2),                     # fi 4
                           lambda: V(8), lambda: V(9),          # fi 5,6
                           lambda: V(10),                       # fi 7
                           lambda: K(0, 3),                     # fi 8
                           lambda: V(11), lambda: V(12),        # fi 9,10
                           lambda: V(13), lambda: V(14),        # fi 11,12
                           lambda: V(15),                       # fi 13
                           lambda: K(1, 0), lambda: Q(1, 0)])   # fi 14,15
            f = emit_unit(1, 0,
                          [lambda: K(1, 1), lambda: K(1, 2),
                           lambda: K(1, 3),
                           lambda: Q(0, 1), lambda: Q(1, 1)], pre=f)
            f = emit_unit(0, 1, [lambda: Q(0, 2), lambda: Q(1, 2)], pre=f)
            f = emit_unit(1, 1, [N, N, N, N]
                          + [lambda q=q: C(q) for q in range(0, 4)]
                          + [lambda: Q(0, 3)], pre=f)
            f = emit_unit(0, 2, [lambda: Q(1, 3), N, N, N]
                          + [lambda q=q: C(q) for q in range(4, 8)], pre=f)
            f = emit_unit(1, 2, [], pre=f)
            f = emit_unit(0, 3, [N, N, N, N]
                          + [lambda q=q: C(q) for q in range(8, 12)], pre=f)
            f = emit_unit(1, 3, [], pre=f)
            f()
            for q in range(12, 16):
                emit_c(q)

    if legalize:
        _legalize_matmul_waits(nc)
    return nc


_NC_CACHE = None


def _get_nc():
    global _NC_CACHE
    if _NC_CACHE is None:
        _NC_CACHE = build_nc()
    return _NC_CACHE


def make_in_maps(inputs):
    x = np.asarray(inputs["x"], dtype=np.float32)
    Wq = np.asarray(inputs["Wq"], dtype=np.float32)
    Wk = np.asarray(inputs["Wk"], dtype=np.float32)
    Wv = np.asarray(inputs["Wv"], dtype=np.float32)
    Wo = np.asarray(inputs["Wo"], dtype=np.float32)
    bq = np.asarray(inputs["bq"], dtype=np.float32)

    f16 = np.float16
    in_maps = []
    for c in range(N_CORES):
        b, g = c // 4, c % 4
        sl = slice(g * E, (g + 1) * E)

        # [1024, n] -> [128, KT, n]
        def kfold(a):
            return np.ascontiguousarray(
                a.reshape(KT, 128, -1).transpose(1, 0, 2))
        in_maps.append({
            "xt": kfold(x[b].T.reshape(D, S)).astype(f16),
            "wq": kfold(Wq[:, sl]).astype(f16),
            "wk": kfold(Wk[:, sl]).astype(f16),
            "wv": kfold(Wv[:, sl]).astype(f16),
            "wo": np.ascontiguousarray(Wo[sl, :]).astype(f16),
            "bq": np.ascontiguousarray(bq[sl]),
        })
    return in_maps


def kernel(x, Wq, bq, Wk, bk, Wv, bv, Wo, bo):
    from concourse.bass_utils import run_bass_kernel_spmd

    x = np.asarray(x, dtype=np.float32)
    Wo = np.asarray(Wo, dtype=np.float32)
    bv = np.asarray(bv, dtype=np.float32)
    bo = np.asarray(bo, dtype=np.float32)

    B = x.shape[0]
    nc = _get_nc()
    in_maps = make_in_maps({
        "x": x, "Wq": Wq, "Wk": Wk, "Wv": Wv, "Wo": Wo, "bq": bq,
    })

    res = run_bass_kernel_spmd(nc, in_maps, core_ids=list(range(N_CORES)))

    bias_total = bo + bv @ Wo  # [D]
    out = np.zeros((B, S, D), dtype=np.float32)
    for c in range(N_CORES):
        out[c // 4] += np.asarray(res.results[c]["y"], dtype=np.float32)
    out += bias_total[None, None, :]
    return out
